# revision 56
# baseline (speedup 1.0000x reference)
"""GCN message-passing Bass kernel for TRN2 (8 cores).

Math: delta = segment_sum(w_e * x[src_e]) @ W^T   (linearity: transform after aggregate)

Sharding: targets split across 8 cores (12500 each). Per core, targets are
degree-sorted and grouped into 128-target blocks; block j gives each of its
128 targets D_j padded edge slots (pad -> weight 0). Blocks are packed into
gathers of <= GATHER_SLOTS slots.

Per gather: ONE InstDMAGatherAnt pulls int8-quantized x rows from a
per-gather DRAM table (the unique x rows of that gather's slot entries,
host-reindexed to int16 ids, 64B payload at 256B row stride -> hits the
DMA per-descriptor floor). The per-row dequant scale is folded into the
host-built edge weights:
  gt[p, s*64:(s+1)*64] = table[idx[s*128+p], :64]
DVE multiplies the gather by per-slot bf16 weights (broadcast AP) into a
bf16 msg tile. PE does the slot reduction: per block, dj accumulating
matmuls (lhsT=slot tile, rhs=identity) sum slot^T into PSUM [64,128]; Act
copies the f32 PSUM to bf16 SBUF; one PE matmul with W^T STATIONARY
(lhsT=wt, rhs=up to 4 blocks' aggT, moving free <= 512) transforms four
blocks at once; Act copies the transposed result [64, 128t] into a
per-gather output tile written to DRAM contiguously in block order (plain
DMA, no scatter). The host transposes back and applies the inverse target
permutation to assemble the final (num_nodes, 64) output.
"""

import math
from contextlib import ExitStack

import numpy as np
import ml_dtypes

import concourse.bass as bass
import concourse.bacc as bacc
import concourse.mybir as mybir
import concourse.tile as tile
from concourse.bass_utils import run_bass_kernel_spmd
from concourse.library_config import mlp as mlp_library

P = 128
N_CORES = 8
D = 64
F32 = mybir.dt.float32
BF16 = mybir.dt.bfloat16
I16 = mybir.dt.int16
I8 = mybir.dt.int8
NP_BF16 = np.dtype(ml_dtypes.bfloat16)

GATHER_SLOTS = 56  # max slots per dma_gather (56*128 = 7168 indices)
TABLE_ROWS = 7168  # per-gather unique-row table size (int16-indexable)


def preprocess(source, target, edge_weights, n_nodes, n_cores=N_CORES,
               src_scale=None):
    """Build per-core gather tables/indices/weights and the shared schedule.

    Returns dict with:
      d_sched: [nblk] per-block slot count (same for all cores)
      gathers: list of (blo, bhi, s0, gsz) gather groups over blocks
      per_core: dict with tables [G,TABLE_ROWS,128] bf16, idx16 [128,S*8] i16,
                w_all [128,S] bf16, perm_pad [nblk*128] i64
      nt, nblk, S
    """
    source = np.asarray(source).astype(np.int64)
    target = np.asarray(target).astype(np.int64)
    edge_weights = np.asarray(edge_weights).astype(np.float32)
    nt = n_nodes // n_cores
    assert nt * n_cores == n_nodes
    nblk = math.ceil(nt / P)
    ntp = nblk * P

    cores = []
    d_sched = np.zeros(nblk, dtype=np.int64)
    for k in range(n_cores):
        lo, hi = k * nt, (k + 1) * nt
        m = (target >= lo) & (target < hi)
        src_k = source[m]
        w_k = edge_weights[m]
        tl_k = target[m] - lo  # local target ids

        deg = np.bincount(tl_k, minlength=nt)
        perm = np.argsort(deg, kind="stable")  # local ids, degree-ascending
        deg_pad = np.concatenate([deg[perm], np.zeros(ntp - nt, dtype=deg.dtype)])
        d_k = deg_pad.reshape(nblk, P).max(axis=1)
        d_sched = np.maximum(d_sched, d_k)
        cores.append(dict(src=src_k, w=w_k, tl=tl_k, deg=deg, perm=perm))

    offs = np.concatenate([[0], np.cumsum(d_sched)]).astype(np.int64)
    S = int(offs[-1])

    # pack blocks into gathers of <= GATHER_SLOTS slots; keep the first and
    # last few gathers single-block so the pipeline head fill and the
    # un-overlapped tail stay short
    single_tail = 2
    gathers = []
    blo = 0
    while blo < nblk:
        bhi = blo
        gsz = 0
        while (bhi < nblk and (bhi == blo or gsz + d_sched[bhi] <= GATHER_SLOTS)
               and not (bhi > blo and bhi >= nblk - single_tail)):
            gsz += int(d_sched[bhi])
            bhi += 1
        gathers.append((blo, bhi, int(offs[blo]), gsz))
        blo = bhi
    G = len(gathers)

    per_core = []
    for k in range(n_cores):
        c = cores[k]
        deg, perm = c["deg"], c["perm"]
        rank = np.empty(nt, dtype=np.int64)
        rank[perm] = np.arange(nt)

        order = np.argsort(c["tl"], kind="stable")
        tls = c["tl"][order]
        srcs = c["src"][order]
        ws = c["w"][order]
        starts = np.cumsum(deg) - deg  # first edge position per target
        eo = np.arange(len(tls)) - starts[tls]  # occurrence index within target
        rr = rank[tls]
        pp = rr & (P - 1)
        bb = rr >> 7
        col = offs[bb] + eo

        entries = np.zeros((P, S), dtype=np.int64)  # pad -> x row 0 (weight 0)
        w_all = np.zeros((P, S), dtype=NP_BF16)
        entries[pp, col] = srcs
        # fold the int8 per-source-row dequant scale into the edge weight
        wsf = ws if src_scale is None else ws * src_scale[srcs].astype(np.float32)
        w_all[pp, col] = wsf.astype(NP_BF16)

        # per-gather unique tables + int16 indices, wrapped for the Q7 layout
        uniq_list = []
        idx16 = np.empty((P, S * 8), dtype=np.int16)
        for gi, (_, _, s0, gsz) in enumerate(gathers):
            ent = entries[:, s0 : s0 + gsz]
            uniq, inv = np.unique(ent, return_inverse=True)
            assert len(uniq) <= TABLE_ROWS
            uniq_list.append(uniq)
            inv = inv.reshape(P, gsz).astype(np.int16)
            iflat = inv.T.reshape(-1)  # position i = s_local*128 + p
            blkcols = np.tile(iflat.reshape(gsz * 8, 16).T, (8, 1))
            idx16[:, s0 * 8 : (s0 + gsz) * 8] = blkcols

        perm_pad = np.full(ntp, -1, dtype=np.int64)
        perm_pad[:nt] = perm
        per_core.append(dict(uniq_list=uniq_list, idx16=idx16, w_all=w_all,
                             perm_pad=perm_pad))

    return dict(d_sched=[int(d) for d in d_sched], S=S, gathers=gathers,
                per_core=per_core, nt=nt, nblk=nblk, G=G)


def _dma_gather(gp, out_ap, in_ap, idxs_ap, num_idxs):
    """InstDMAGatherAnt with a 128B payload at 256B row stride (elem_size=64
    bf16, stride_bytes_256=1). bass.dma_gather asserts elem%256B, but the Q7
    ucode handles 128B payloads (verified on HW); construct directly."""
    _in_ap = gp.lower_ap_dma(in_ap, for_custom_bir_dma=True)
    _idxs_ap = gp.lower_ap(idxs_ap)
    _out_ap = gp.lower_ap(out_ap)
    return gp.add_instruction(
        mybir.InstDMAGatherAnt(
            name=gp.bass.get_next_instruction_name(),
            ins=[*_in_ap, _idxs_ap, gp.lower_val_access(gp.to_reg(num_idxs))],
            outs=[_out_ap],
            transpose=False,
            num_idxs=num_idxs,
            elem_size=D,
            stride_bytes_256=1,
            gen_mode=0,
            single_packet=False,
            queue_num=0,
            sbuf_tokens_per_rank=0,
            sbuf_free_dim_per_rank=0,
            sbuf_free_dim_pad_per_rank=0,
            sbuf_byte_offset=0,
        )
    )


def build_nc(pp, n_nodes, bufs=6, out_bf16=True, psum_bufs=4, stages=3,
             aggT_on_dve=False, pair_transpose=False, act_slots=0,
             ipool_bufs=None):
    # stages: 1=gather only, 2=+mult, 3=full (ablation knob for timing)
    d_sched, S, nblk, gathers = pp["d_sched"], pp["S"], pp["nblk"], pp["gathers"]
    nc = bacc.Bacc("TRN2", target_bir_lowering=False, debug=False)
    # int8 rows, padded to a 256B stride; payload = first 64 bytes
    tabs = [nc.dram_tensor(f"xg{gi}", [TABLE_ROWS, 4 * D], I8, kind="ExternalInput")
            for gi in range(len(gathers))]
    wt_t = nc.dram_tensor("wT", [D, D], BF16, kind="ExternalInput")
    idx_t = nc.dram_tensor("idx", [P, S * 8], I16, kind="ExternalInput")
    wgt_t = nc.dram_tensor("wgt", [P, S], BF16, kind="ExternalInput")
    eye_t = nc.dram_tensor("eye", [P, P], BF16, kind="ExternalInput")
    out_dt = BF16 if out_bf16 else F32
    # transposed output layout: [64 features, nblk*128 block-order targets]
    out_t = nc.dram_tensor("out", [D, nblk * P], out_dt, kind="ExternalOutput")

    with tile.TileContext(nc) as tc, ExitStack() as ctx:
        nc.gpsimd.load_library(mlp_library)
        const = ctx.enter_context(tc.tile_pool(name="const", bufs=1))
        gpool = ctx.enter_context(tc.tile_pool(name="gather", bufs=bufs))
        mpool = ctx.enter_context(tc.tile_pool(name="msg", bufs=bufs))
        tpool = ctx.enter_context(tc.tile_pool(name="aggT", bufs=8))
        dpool = ctx.enter_context(tc.tile_pool(name="delta", bufs=bufs))
        psa_bufs, psb_bufs = (psum_bufs if isinstance(psum_bufs, (tuple, list))
                              else (psum_bufs, psum_bufs))
        psA = ctx.enter_context(tc.tile_pool(name="psA", bufs=psa_bufs, space="PSUM"))
        psB = ctx.enter_context(tc.tile_pool(name="psB", bufs=psb_bufs, space="PSUM"))

        ident = const.tile([P, P], BF16)
        nc.sync.dma_start(out=ident[:], in_=eye_t.ap())
        wt_sb = const.tile([D, D], BF16)
        nc.sync.dma_start(out=wt_sb[:], in_=wt_t.ap())
        # per-gather idx slices through a buffered pool: loads beyond the
        # pipeline depth cannot enter the DMA queue early, so gather 0's
        # transfer is not stuck behind bulk idx loading
        ipool = ctx.enter_context(tc.tile_pool(name="idx", bufs=ipool_bufs or bufs))
        wgt_sb = const.tile([P, S], BF16)
        nc.sync.dma_start(out=wgt_sb[:], in_=wgt_t.ap())

        # Prime engines on the upfront loads so per-block instructions carry
        # at most one sync wait each (SEQ instruction structs encode one).
        prime = const.tile([P, 1], BF16)
        nc.vector.tensor_copy(out=prime[:], in_=wgt_sb[:, :1])
        prime2 = const.tile([P, 1], BF16)
        nc.scalar.copy(out=prime2[:], in_=ident[:, :1])
        prime_ps = psA.tile([D, P], F32, tag="agg")
        nc.tensor.matmul(out=prime_ps[:], lhsT=ident[:, :D], rhs=ident[:],
                         start=True, stop=True)

        for gi, (blo, bhi, s0, gsz) in enumerate(gathers):
            it = ipool.tile([P, gsz * 8], I16, tag="i")
            nc.sync.dma_start(out=it[:], in_=idx_t.ap()[:, s0 * 8 : (s0 + gsz) * 8])
            gt = gpool.tile([P, gsz * D], I8, tag="g")
            _dma_gather(
                nc.gpsimd,
                gt[:].rearrange("p (c e) -> p c e", e=D),
                tabs[gi].ap(),
                it[:],
                gsz * P,
            )
            if stages < 2:
                continue
            msg = mpool.tile([P, gsz * D], BF16, tag="m")
            # last act_slots slots multiplied on the Activation engine
            # (per-slot per-partition scale); the rest in one DVE op
            ka = min(act_slots, gsz - 1)
            mv = gsz - ka
            nc.vector.tensor_tensor(
                out=msg[:, : mv * D].rearrange("p (d o) -> p d o", o=D),
                in0=gt[:, : mv * D].rearrange("p (d o) -> p d o", o=D),
                in1=wgt_sb[:, s0 : s0 + mv].to_broadcast([P, mv, D]),
                op=mybir.AluOpType.mult,
            )
            for s in range(mv, gsz):
                nc.scalar.mul(
                    out=msg[:, s * D : (s + 1) * D],
                    in_=gt[:, s * D : (s + 1) * D],
                    mul=wgt_sb[:, s0 + s : s0 + s + 1],
                )
            if stages < 3:
                continue

            ng = bhi - blo
            # output kept TRANSPOSED on device: dgrp [64 o, ng*128 t]; the
            # host transposes back. W is the stationary matmul operand so one
            # matmul transforms up to 4 blocks (rhs free dim <= 512).
            dgrp = dpool.tile([D, ng * P], out_dt, tag="d")
            lo = 0
            j = blo
            while j < bhi:
                nb = min(4, bhi - j)  # blocks in this W-matmul batch
                aggT = tpool.tile([D, nb * P], BF16, tag="aT")
                for b in range(nb):
                    dj = d_sched[j + b]
                    agg_ps = psA.tile([D, P], F32, tag="agg")
                    for dd in range(dj):
                        nc.tensor.matmul(
                            out=agg_ps[:],
                            lhsT=msg[:, (lo + dd) * D : (lo + dd + 1) * D],
                            rhs=ident[:],
                            start=(dd == 0),
                            stop=(dd == dj - 1),
                        )
                    nc.scalar.copy(out=aggT[:, b * P : (b + 1) * P], in_=agg_ps[:])
                    lo += dj
                dT_ps = psB.tile([D, nb * P], F32, tag="mm")
                nc.tensor.matmul(out=dT_ps[:], lhsT=wt_sb[:], rhs=aggT[:],
                                 start=True, stop=True)
                nc.scalar.copy(
                    out=dgrp[:, (j - blo) * P : (j - blo + nb) * P], in_=dT_ps[:])
                j += nb

            nc.sync.dma_start(out=out_t.ap()[:, blo * P : bhi * P], in_=dgrp[:])
    nc.compile()
    return nc


def run_gcn(x, W, edge_weights, source, target, num_nodes, trace=False, bufs=6):
    """Full-input host entry: preprocess, build, run on 8 cores, assemble output."""
    n_nodes = int(num_nodes)
    x_f32 = np.ascontiguousarray(np.asarray(x, dtype=np.float32))
    # int8 row quantization; the per-row dequant scale folds into edge weights
    row_absmax = np.abs(x_f32).max(axis=1)
    scale = np.where(row_absmax > 0, row_absmax / 127.0, 1.0).astype(np.float32)
    x_i8 = np.clip(np.rint(x_f32 / scale[:, None]), -127, 127).astype(np.int8)

    pp = preprocess(source, target, edge_weights, n_nodes, src_scale=scale)
    nc = build_nc(pp, n_nodes, bufs=bufs)
    wt_np = np.ascontiguousarray(np.asarray(W, dtype=np.float32).T).astype(NP_BF16)
    eye_np = np.eye(P, dtype=np.float32).astype(NP_BF16)
    in_maps = []
    for k in range(N_CORES):
        pc = pp["per_core"][k]
        im = {"wT": wt_np, "eye": eye_np, "idx": pc["idx16"], "wgt": pc["w_all"]}
        for gi, uniq in enumerate(pc["uniq_list"]):
            tab = np.zeros((TABLE_ROWS, 4 * D), dtype=np.int8)
            tab[: len(uniq), :D] = x_i8[uniq]
            im[f"xg{gi}"] = tab
        in_maps.append(im)
    res = run_bass_kernel_spmd(nc, in_maps, core_ids=list(range(N_CORES)), trace=trace)

    nt, nblk = pp["nt"], pp["nblk"]
    out = np.empty((n_nodes, D), dtype=np.float32)
    for k in range(N_CORES):
        raw = np.asarray(res.results[k]["out"], dtype=np.float32)  # [64, nblk*128]
        cube = raw.T  # [nblk*128, 64], row j*128+p = (block j, partition p)
        perm_pad = pp["per_core"][k]["perm_pad"]
        valid = perm_pad >= 0
        out[k * nt + perm_pad[valid]] = cube[valid]
    return out, res


def kernel(**inputs) -> np.ndarray:
    """Harness entry: full unsharded inputs -> full (num_nodes, 64) output."""
    out, _ = run_gcn(
        np.asarray(inputs["x"]),
        np.asarray(inputs["W"]),
        np.asarray(inputs["edge_weights"]),
        np.asarray(inputs["source"]),
        np.asarray(inputs["target"]),
        int(inputs["num_nodes"]),
        trace=False,
    )
    return out
